# revision 36
# baseline (speedup 1.0000x reference)
"""Bass TRN2 kernel for nn_ACCLoss (histogram_binning), 8-core data parallel.

Each of the 8 NeuronCores processes B/8 = 512 samples:
  - streams entity_vectors [512, 64, 256] f32 from HBM as bf16 (cast in DMA),
  - computes squared distances ||v[b,e,:] - predicted[b,:]||^2 via a
    broadcast-subtract (VectorE) + in-place square (ScalarE) + tree-add
    reduction (VectorE),
  - derives per-sample (is_known, has_neg) categories and per-threshold
    incorrect flags from entity_mask (all comparisons done in squared-distance
    space; thresholds squared),
  - reduces the per-sample 0/1 payload columns across the 128-sample
    partition axis with a ones-vector matmul accumulated in PSUM.
Host side sums the 8 per-core count vectors (the "all-reduce" of the
sharding hint — 19 scalars per core — done in numpy at unshard time).
"""

import numpy as np

import concourse.bass as bass
import concourse.mybir as mybir
import concourse.tile as tile
from concourse.vector_clock import ScopedClock

B, E, D = 4096, 64, 256
NCORES = 8
BL = B // NCORES          # samples per core
P = 128                   # samples per group (partition dim)
G = BL // P               # groups per core
THR2 = (0.25, 1.0, 2.25, 4.0, 9.0)   # squared thresholds [0.5,1,1.5,2,3]^2
T = len(THR2)
NC_OUT = 2 + 3 * T        # rhs cols: [1-hn, hn, hn*g1, (1-hn)*g2, hn*g3]
BIGF = float(2 ** 20)

f32 = mybir.dt.float32
bf16 = mybir.dt.bfloat16
i32 = mybir.dt.int32
Alu = mybir.AluOpType
Act = mybir.ActivationFunctionType
AX = mybir.AxisListType


_MAXW = 1  # max sync-waits this walrus build accepts per instruction


class _SplitDrainTC(tile.TileContext):
    """This container's walrus build rejects instructions carrying more
    than one sync-wait ("Too many sync wait commands"); split extra waits
    onto same-engine NoOp carriers emitted just before the instruction."""

    def _commit_and_lower(self, inst, original_block, old_bb_map, bb_to_exit_bb):
        si = getattr(inst, "sync_info", None)
        eng = getattr(inst, "engine", None)
        if si is not None and eng is not None and len(si.on_wait) > _MAXW:
            ow = list(si.on_wait)
            head, tail = ow[:-_MAXW], ow[-_MAXW:]
            while head:
                chunk, head = head[:_MAXW], head[_MAXW:]
                nop = mybir.InstNoOp(
                    name=self.nc.get_next_instruction_name(),
                    sync_info=mybir.SyncInfo(on_wait=chunk, on_update=[]),
                    bass_nofuse=True,
                    engine=eng,
                )
                self._commit_instruction(nop)
            inst.sync_info = mybir.SyncInfo(on_wait=tail, on_update=si.on_update)
        return super()._commit_and_lower(inst, original_block, old_bb_map,
                                         bb_to_exit_bb)

    def _drain_and_barrier(self, tick_clock, wait_clock):
        drain_inst = self.nc.sync.drain()
        wait_clock.add_sem_waits(
            drain_inst.ins, ScopedClock({None: tick_clock.global_clock})
        )
        si = drain_inst.ins.sync_info
        ow = list(si.on_wait) if si is not None else []
        if len(ow) > 1:
            drain_inst.ins.sync_info = mybir.SyncInfo(
                on_wait=ow[:1], on_update=si.on_update
            )
            for w in ow[1:]:
                d = self.nc.sync.drain()
                d.ins.sync_info = mybir.SyncInfo(on_wait=[w], on_update=[])
        self.nc.all_engine_barrier()
        assert self.sems is not None
        popped = self.nc._tile_sem_poison_stack.pop()
        assert popped is self._sem_poison
        self.nc.clear_and_free_semaphores(list(self.sems.allocated().values()))
        self.nc.all_engine_barrier()


def build_nc():
    nc = bass.Bass("TRN2", target_bir_lowering=False, debug=False,
                   num_devices=NCORES)
    pred = nc.dram_tensor("predicted", [BL, D], f32, kind="ExternalInput")
    ev = nc.dram_tensor("entity_vectors", [BL, E, D], f32, kind="ExternalInput")
    em = nc.dram_tensor("entity_mask", [BL, E, 2], i32, kind="ExternalInput")
    out = nc.dram_tensor("out", [2, NC_OUT], f32, kind="ExternalOutput")

    with _SplitDrainTC(nc) as tc, \
         tc.tile_pool(name="v", bufs=8) as vpool, \
         tc.tile_pool(name="diff", bufs=4) as dpool, \
         tc.tile_pool(name="scr", bufs=2) as spool, \
         tc.tile_pool(name="small", bufs=2) as smpool, \
         tc.tile_pool(name="const", bufs=1) as cpool, \
         tc.tile_pool(name="psum", bufs=1, space="PSUM") as ppool:

        # ---- one-time constants
        iota_i = cpool.tile([P, G * E], i32, tag="iota_i")
        nc.gpsimd.iota(iota_i[:].rearrange("p (g e) -> p g e", g=G),
                       [[0, G], [1, E]], channel_multiplier=0)
        iota_f = cpool.tile([P, G * E], f32, tag="iota_f")
        nc.vector.tensor_copy(iota_f[:], iota_i[:])
        iota_mb = cpool.tile([P, G * E], f32, tag="iota_mb")   # iota - BIG
        nc.vector.tensor_scalar(iota_mb[:], iota_f[:], -BIGF, None, Alu.add)
        thr2 = cpool.tile([P, T], f32, tag="thr2")
        for i, t2 in enumerate(THR2):
            nc.gpsimd.memset(thr2[:, i:i + 1], t2)
        ones = cpool.tile([P, 1], f32, tag="ones")
        nc.gpsimd.memset(ones[:], 1.0)
        big1 = cpool.tile([P, 1], f32, tag="big1")
        nc.gpsimd.memset(big1[:], BIGF)

        acc = ppool.tile([2, NC_OUT], f32, tag="acc")

        # predicted and mask for ALL groups upfront as two contiguous DMAs
        # (a strided per-group mask DMA generates tiny descriptors that clog
        # the SDMA engines for tens of us)
        p_all = cpool.tile([P, G * D], bf16, tag="p_all")
        nc.gpsimd.dma_start(p_all[:].rearrange("p (g d) -> p g d", g=G),
                            pred[:].rearrange("(g p) d -> p g d", g=G))
        m_all = cpool.tile([P, G * E * 2], i32, tag="m_all")
        nc.sync.dma_start(m_all[:].rearrange("p (g e c) -> p g e c", g=G, c=2),
                          em[:].rearrange("(g p) e c -> p g e c", g=G))
        m4 = m_all[:].rearrange("p (g e c) -> p g e c", g=G, c=2)

        # batched mask path for ALL groups (independent of the distance data)
        mkf_all = cpool.tile([P, G * E], f32, tag="mkf_all")
        nc.vector.tensor_copy(mkf_all[:].rearrange("p (g e) -> p g e", g=G),
                              m4[:, :, :, 0])
        sel_all = cpool.tile([P, G * E], f32, tag="sel_all")
        nc.vector.tensor_tensor(sel_all[:], mkf_all[:], iota_mb[:], Alu.mult)
        rm_all = cpool.tile([P, G], f32, tag="rm_all")
        nc.vector.tensor_reduce(rm_all[:],
                                sel_all[:].rearrange("p (g e) -> p g e", g=G),
                                axis=AX.X, op=Alu.min)
        gt_all = cpool.tile([P, G * E], f32, tag="gt_all")
        nc.vector.tensor_tensor(
            gt_all[:].rearrange("p (g e) -> p g e", g=G),
            iota_mb[:].rearrange("p (g e) -> p g e", g=G),
            rm_all[:][:, :, None].broadcast_to([P, G, E]), Alu.is_gt)
        neg_all = cpool.tile([P, G * E], f32, tag="neg_all")
        nc.vector.tensor_tensor(neg_all[:], gt_all[:], mkf_all[:], Alu.mult)
        hn_all = cpool.tile([P, G], f32, tag="hn_all")
        nc.vector.tensor_reduce(hn_all[:],
                                neg_all[:].rearrange("p (g e) -> p g e", g=G),
                                axis=AX.X, op=Alu.max)

        NQ = 4           # work items per group
        QE = E // NQ     # entities per work item
        HE = E // 2

        def load_q(g, q):
            r0 = g * P
            v = vpool.tile([P, QE * D], bf16, tag="v")
            src = ev[r0:r0 + P, q * QE:(q + 1) * QE, :].rearrange(
                "p e d -> p (e d)")
            nc.gpsimd.dma_start(v[:], src)
            return v

        def subtract(g, q, v, df):
            # quarter subtract into its slice of a half-group diff tile,
            # then in-place square of that slice on ScalarE
            sl = df[:, (q % 2) * QE * D:((q % 2) + 1) * QE * D]
            v3 = v[:].rearrange("p (e d) -> p e d", e=QE)
            d3 = sl.rearrange("p (e d) -> p e d", e=QE)
            p3 = p_all[:, g * D:(g + 1) * D][:, None, :].broadcast_to([P, QE, D])
            nc.vector.tensor_tensor(d3, v3, p3, Alu.subtract)
            nc.scalar.activation(sl, sl, Act.Square)

        def tree_h(df, dist2, h):
            scrA = spool.tile([P, HE * (D // 2)], bf16, tag="scrA")
            scrB = spool.tile([P, HE * (D // 4)], bf16, tag="scrB")

            def lvl(src_ap, dst_ap, w):
                s3 = src_ap.rearrange("p (e w) -> p e w", e=HE)
                half = w // 2
                nc.vector.tensor_tensor(
                    dst_ap.rearrange("p (e h) -> p e h", e=HE),
                    s3[:, :, 0:half], s3[:, :, half:w], Alu.add)

            cur, w = df[:], D
            use_a = True
            while w > 2:
                dst_t = scrA if use_a else scrB
                dst = dst_t[:, 0:HE * (w // 2)]
                lvl(cur, dst, w)
                cur, w, use_a = dst, w // 2, not use_a
            lvl(cur, dist2[:, h * HE:(h + 1) * HE], 2)

        def epilogue(g, dist2):
            neg = neg_all[:, g * E:(g + 1) * E]
            hn = hn_all[:, g:g + 1]
            # min over negatives of dist2:  max(neg*BIG - dist2) = BIG - mn2
            sel2 = smpool.tile([P, E], f32, tag="sel2")
            nc.vector.scalar_tensor_tensor(sel2[:], neg, BIGF, dist2[:],
                                           Alu.mult, Alu.subtract)
            mx = smpool.tile([P, 1], f32, tag="mx")
            nc.vector.reduce_max(mx[:], sel2[:], axis=AX.X)
            mn2 = smpool.tile([P, 1], f32, tag="mn2")
            nc.scalar.activation(mn2[:], mx[:], Act.Identity, bias=big1[:], scale=-1.0)

            k = mkf_all[:, g * E:g * E + 1]
            pos2 = dist2[:, 0:1]
            # lhsT = [1-k, k]; rhs = [1-hn, hn, hn*g1, (1-hn)*g2, hn*g3]
            # PE outer product then gives every count/incorrect cell.
            lhsT = smpool.tile([P, 2], f32, tag="lhsT")
            nc.scalar.activation(lhsT[:, 0:1], k, Act.Identity, bias=ones[:],
                                 scale=-1.0)
            nc.scalar.copy(lhsT[:, 1:2], k)
            rhs = smpool.tile([P, 2 + 3 * T], f32, tag="rhs")
            nc.scalar.activation(rhs[:, 0:1], hn, Act.Identity, bias=ones[:],
                                 scale=-1.0)
            nc.scalar.copy(rhs[:, 1:2], hn)
            # g1[t] = (mn < t) = (thr2 is_gt mn2);  weighted by hn
            nc.vector.tensor_scalar(rhs[:, 2:2 + T], thr2[:], mn2[:],
                                    rhs[:, 1:2], Alu.is_gt, Alu.mult)
            # g2[t] = (t < pos) = (thr2 is_lt pos2);  weighted by (1-hn)
            nc.vector.tensor_scalar(rhs[:, 2 + T:2 + 2 * T], thr2[:], pos2,
                                    rhs[:, 0:1], Alu.is_lt, Alu.mult)
            # g3[t] = (min(mn2, t2) < pos2);  weighted by hn
            g3 = smpool.tile([P, T], f32, tag="g3")
            nc.vector.tensor_scalar(g3[:], thr2[:], mn2[:], pos2,
                                    Alu.min, Alu.is_lt)
            nc.vector.tensor_scalar(rhs[:, 2 + 2 * T:2 + 3 * T], g3[:],
                                    rhs[:, 1:2], None, Alu.mult)
            nc.tensor.matmul(acc[:], lhsT[:], rhs[:],
                             start=(g == 0), stop=(g == G - 1))

        # ---- software-pipelined emission (quarter loads, half trees)
        vs = {}
        diffs = {}

        def stage_subtracts(g):
            dfa = dpool.tile([P, HE * D], bf16, tag="diff")
            dfb = dpool.tile([P, HE * D], bf16, tag="diff")
            for q in range(NQ):
                subtract(g, q, vs[(g, q)], dfa if q < 2 else dfb)
            diffs[g] = (dfa, dfb)

        for g in range(min(2, G)):
            for q in range(NQ):
                vs[(g, q)] = load_q(g, q)
        for g in range(min(2, G)):
            stage_subtracts(g)
        for g in range(G):
            if g + 2 < G:
                for q in range(NQ):
                    vs[(g + 2, q)] = load_q(g + 2, q)
            dist2 = smpool.tile([P, E], f32, tag="dist2")
            tree_h(diffs[g][0], dist2, 0)
            tree_h(diffs[g][1], dist2, 1)
            epilogue(g, dist2)
            if g + 2 < G:
                stage_subtracts(g + 2)

        outsb = smpool.tile([2, 2 + 3 * T], f32, tag="outsb")
        nc.vector.tensor_copy(outsb[:], acc[:])
        nc.sync.dma_start(out[:], outsb[:])

    return nc


_CACHE = {}


def _run(in_maps, trace=False):
    from concourse.bass_utils import run_bass_kernel_spmd
    if "nc" not in _CACHE:
        _CACHE["nc"] = build_nc()
    return run_bass_kernel_spmd(_CACHE["nc"], in_maps,
                                core_ids=list(range(NCORES)), trace=trace)


def shard_inputs(predicted, entity_vectors, entity_mask):
    in_maps = []
    for c in range(NCORES):
        s = slice(c * BL, (c + 1) * BL)
        in_maps.append({
            "predicted": np.ascontiguousarray(predicted[s], dtype=np.float32),
            "entity_vectors": np.ascontiguousarray(entity_vectors[s],
                                                   dtype=np.float32),
            "entity_mask": np.ascontiguousarray(entity_mask[s], dtype=np.int32),
        })
    return in_maps


def unshard(results):
    total = np.zeros((2, NC_OUT), dtype=np.float64)
    for r in results:
        total += r["out"].astype(np.float64)
    counts = np.rint(total[:, 0:2]).astype(np.int32)
    incorrect = np.zeros((T, 2, 2), dtype=np.int32)
    incorrect[:, 0, 1] = np.rint(total[0, 2:2 + T]).astype(np.int32)
    incorrect[:, 1, 0] = np.rint(total[1, 2 + T:2 + 2 * T]).astype(np.int32)
    incorrect[:, 1, 1] = np.rint(total[1, 2 + 2 * T:2 + 3 * T]).astype(np.int32)
    return counts, incorrect


def kernel(predicted, entity_vectors, entity_mask):
    res = _run(shard_inputs(predicted, entity_vectors, entity_mask))
    return unshard(res.results)


# revision 40
# speedup vs baseline: 1.0797x; 1.0797x over previous
"""Bass TRN2 kernel for nn_ACCLoss (histogram_binning), 8-core data parallel.

Each of the 8 NeuronCores processes B/8 = 512 samples:
  - streams entity_vectors [512, 64, 256] f32 from HBM as bf16 (cast in DMA),
  - computes squared distances ||v[b,e,:] - predicted[b,:]||^2 via a
    broadcast-subtract (VectorE) + in-place square (ScalarE) + tree-add
    reduction (VectorE),
  - derives per-sample (is_known, has_neg) categories and per-threshold
    incorrect flags from entity_mask (all comparisons done in squared-distance
    space; thresholds squared),
  - reduces the per-sample 0/1 payload columns across the 128-sample
    partition axis with a ones-vector matmul accumulated in PSUM.
Host side sums the 8 per-core count vectors (the "all-reduce" of the
sharding hint — 19 scalars per core — done in numpy at unshard time).
"""

import numpy as np

import concourse.bass as bass
import concourse.mybir as mybir
import concourse.tile as tile
from concourse.vector_clock import ScopedClock

B, E, D = 4096, 64, 256
NCORES = 8
BL = B // NCORES          # samples per core
P = 128                   # samples per group (partition dim)
G = BL // P               # groups per core
THR2 = (0.25, 1.0, 2.25, 4.0, 9.0)   # squared thresholds [0.5,1,1.5,2,3]^2
T = len(THR2)
NC_OUT = 2 + 3 * T        # rhs cols: [1-hn, hn, hn*g1, (1-hn)*g2, hn*g3]
BIGF = float(2 ** 20)

f32 = mybir.dt.float32
bf16 = mybir.dt.bfloat16
i32 = mybir.dt.int32
Alu = mybir.AluOpType
Act = mybir.ActivationFunctionType
AX = mybir.AxisListType


_MAXW = 1  # max sync-waits this walrus build accepts per instruction


class _SplitDrainTC(tile.TileContext):
    """This container's walrus build rejects instructions carrying more
    than one sync-wait ("Too many sync wait commands"); split extra waits
    onto same-engine NoOp carriers emitted just before the instruction."""

    def _commit_and_lower(self, inst, original_block, old_bb_map, bb_to_exit_bb):
        si = getattr(inst, "sync_info", None)
        eng = getattr(inst, "engine", None)
        if si is not None and eng is not None and len(si.on_wait) > _MAXW:
            ow = list(si.on_wait)
            head, tail = ow[:-_MAXW], ow[-_MAXW:]
            while head:
                chunk, head = head[:_MAXW], head[_MAXW:]
                nop = mybir.InstNoOp(
                    name=self.nc.get_next_instruction_name(),
                    sync_info=mybir.SyncInfo(on_wait=chunk, on_update=[]),
                    bass_nofuse=True,
                    engine=eng,
                )
                self._commit_instruction(nop)
            inst.sync_info = mybir.SyncInfo(on_wait=tail, on_update=si.on_update)
        return super()._commit_and_lower(inst, original_block, old_bb_map,
                                         bb_to_exit_bb)

    def _drain_and_barrier(self, tick_clock, wait_clock):
        drain_inst = self.nc.sync.drain()
        wait_clock.add_sem_waits(
            drain_inst.ins, ScopedClock({None: tick_clock.global_clock})
        )
        si = drain_inst.ins.sync_info
        ow = list(si.on_wait) if si is not None else []
        if len(ow) > 1:
            drain_inst.ins.sync_info = mybir.SyncInfo(
                on_wait=ow[:1], on_update=si.on_update
            )
            for w in ow[1:]:
                d = self.nc.sync.drain()
                d.ins.sync_info = mybir.SyncInfo(on_wait=[w], on_update=[])
        self.nc.all_engine_barrier()
        assert self.sems is not None
        popped = self.nc._tile_sem_poison_stack.pop()
        assert popped is self._sem_poison
        self.nc.clear_and_free_semaphores(list(self.sems.allocated().values()))
        self.nc.all_engine_barrier()


def build_nc():
    nc = bass.Bass("TRN2", target_bir_lowering=False, debug=False,
                   num_devices=NCORES)
    pred = nc.dram_tensor("predicted", [BL, D], f32, kind="ExternalInput")
    ev = nc.dram_tensor("entity_vectors", [BL, E, D], f32, kind="ExternalInput")
    em = nc.dram_tensor("entity_mask", [BL, E, 2], i32, kind="ExternalInput")
    out = nc.dram_tensor("out", [2, NC_OUT], f32, kind="ExternalOutput")

    with _SplitDrainTC(nc) as tc, \
         tc.tile_pool(name="v", bufs=8) as vpool, \
         tc.tile_pool(name="diff", bufs=4) as dpool, \
         tc.tile_pool(name="scr", bufs=2) as spool, \
         tc.tile_pool(name="small", bufs=2) as smpool, \
         tc.tile_pool(name="const", bufs=1) as cpool, \
         tc.tile_pool(name="psum", bufs=1, space="PSUM") as ppool:

        acc = ppool.tile([2, NC_OUT], f32, tag="acc")

        # predicted for all groups: one contiguous cast-DMA, queued first
        p_all = cpool.tile([P, G * D], bf16, tag="p_all")
        nc.gpsimd.dma_start(p_all[:].rearrange("p (g d) -> p g d", g=G),
                            pred[:].rearrange("(g p) d -> p g d", g=G))

        NQ = 4           # work items per group
        QE = E // NQ     # entities per work item
        HE = E // 2

        def emit_consts():
            # emitted AFTER the first v DMAs so the Q7 pushes those
            # descriptors before it runs iota; memsets go on DVE
            iota_i = cpool.tile([P, G * E], i32, tag="iota_i")
            nc.gpsimd.iota(iota_i[:].rearrange("p (g e) -> p g e", g=G),
                           [[0, G], [1, E]], channel_multiplier=0)
            iota_f = cpool.tile([P, G * E], f32, tag="iota_f")
            nc.vector.tensor_copy(iota_f[:], iota_i[:])
            iota_mb = cpool.tile([P, G * E], f32, tag="iota_mb")  # iota - BIG
            nc.vector.tensor_scalar(iota_mb[:], iota_f[:], -BIGF, None, Alu.add)
            thr2 = cpool.tile([P, T], f32, tag="thr2")
            for i, t2 in enumerate(THR2):
                nc.vector.memset(thr2[:, i:i + 1], t2)
            ones = cpool.tile([P, 1], f32, tag="ones")
            nc.vector.memset(ones[:], 1.0)
            big1 = cpool.tile([P, 1], f32, tag="big1")
            nc.vector.memset(big1[:], BIGF)

            # mask for all groups: one contiguous DMA on the HWDGE path
            m_all = cpool.tile([P, G * E * 2], i32, tag="m_all")
            nc.sync.dma_start(
                m_all[:].rearrange("p (g e c) -> p g e c", g=G, c=2),
                em[:].rearrange("(g p) e c -> p g e c", g=G))
            m4 = m_all[:].rearrange("p (g e c) -> p g e c", g=G, c=2)

            # batched mask path for ALL groups (independent of distance data)
            mkf_all = cpool.tile([P, G * E], f32, tag="mkf_all")
            nc.vector.tensor_copy(mkf_all[:].rearrange("p (g e) -> p g e", g=G),
                                  m4[:, :, :, 0])
            sel_all = cpool.tile([P, G * E], f32, tag="sel_all")
            nc.vector.tensor_tensor(sel_all[:], mkf_all[:], iota_mb[:], Alu.mult)
            rm_all = cpool.tile([P, G], f32, tag="rm_all")
            nc.vector.tensor_reduce(
                rm_all[:], sel_all[:].rearrange("p (g e) -> p g e", g=G),
                axis=AX.X, op=Alu.min)
            gt_all = cpool.tile([P, G * E], f32, tag="gt_all")
            nc.vector.tensor_tensor(
                gt_all[:].rearrange("p (g e) -> p g e", g=G),
                iota_mb[:].rearrange("p (g e) -> p g e", g=G),
                rm_all[:][:, :, None].broadcast_to([P, G, E]), Alu.is_gt)
            neg_all = cpool.tile([P, G * E], f32, tag="neg_all")
            nc.vector.tensor_tensor(neg_all[:], gt_all[:], mkf_all[:], Alu.mult)
            hn_all = cpool.tile([P, G], f32, tag="hn_all")
            nc.vector.tensor_reduce(
                hn_all[:], neg_all[:].rearrange("p (g e) -> p g e", g=G),
                axis=AX.X, op=Alu.max)
            return thr2, ones, big1, mkf_all, neg_all, hn_all

        def load_q(g, q):
            r0 = g * P
            v = vpool.tile([P, QE * D], bf16, tag="v")
            src = ev[r0:r0 + P, q * QE:(q + 1) * QE, :].rearrange(
                "p e d -> p (e d)")
            nc.gpsimd.dma_start(v[:], src)
            return v

        def subtract(g, q, v, df):
            # quarter subtract into its slice of a half-group diff tile,
            # then in-place square of that slice on ScalarE
            sl = df[:, (q % 2) * QE * D:((q % 2) + 1) * QE * D]
            v3 = v[:].rearrange("p (e d) -> p e d", e=QE)
            d3 = sl.rearrange("p (e d) -> p e d", e=QE)
            p3 = p_all[:, g * D:(g + 1) * D][:, None, :].broadcast_to([P, QE, D])
            nc.vector.tensor_tensor(d3, v3, p3, Alu.subtract)
            nc.scalar.activation(sl, sl, Act.Square)

        def tree(df_ap, dist2, e0, ne):
            # reduce [P, ne, 256] -> dist2[:, e0:e0+ne]
            scrA = spool.tile([P, HE * (D // 2)], bf16, tag="scrA")
            scrB = spool.tile([P, HE * (D // 4)], bf16, tag="scrB")

            def lvl(src_ap, dst_ap, w):
                s3 = src_ap.rearrange("p (e w) -> p e w", e=ne)
                half = w // 2
                nc.vector.tensor_tensor(
                    dst_ap.rearrange("p (e h) -> p e h", e=ne),
                    s3[:, :, 0:half], s3[:, :, half:w], Alu.add)

            cur, w = df_ap, D
            use_a = True
            while w > 2:
                dst_t = scrA if use_a else scrB
                dst = dst_t[:, 0:ne * (w // 2)]
                lvl(cur, dst, w)
                cur, w, use_a = dst, w // 2, not use_a
            lvl(cur, dist2[:, e0:e0 + ne], 2)

        def epilogue(g, dist2):
            neg = neg_all[:, g * E:(g + 1) * E]
            hn = hn_all[:, g:g + 1]
            # min over negatives of dist2:  max(neg*BIG - dist2) = BIG - mn2
            sel2 = smpool.tile([P, E], f32, tag="sel2")
            nc.vector.scalar_tensor_tensor(sel2[:], neg, BIGF, dist2[:],
                                           Alu.mult, Alu.subtract)
            mx = smpool.tile([P, 1], f32, tag="mx")
            nc.vector.reduce_max(mx[:], sel2[:], axis=AX.X)
            mn2 = smpool.tile([P, 1], f32, tag="mn2")
            nc.scalar.activation(mn2[:], mx[:], Act.Identity, bias=big1[:], scale=-1.0)

            k = mkf_all[:, g * E:g * E + 1]
            pos2 = dist2[:, 0:1]
            # lhsT = [1-k, k]; rhs = [1-hn, hn, hn*g1, (1-hn)*g2, hn*g3]
            # PE outer product then gives every count/incorrect cell.
            lhsT = smpool.tile([P, 2], f32, tag="lhsT")
            nc.scalar.activation(lhsT[:, 0:1], k, Act.Identity, bias=ones[:],
                                 scale=-1.0)
            nc.scalar.copy(lhsT[:, 1:2], k)
            rhs = smpool.tile([P, 2 + 3 * T], f32, tag="rhs")
            nc.scalar.activation(rhs[:, 0:1], hn, Act.Identity, bias=ones[:],
                                 scale=-1.0)
            nc.scalar.copy(rhs[:, 1:2], hn)
            # g1[t] = (mn < t) = (thr2 is_gt mn2);  weighted by hn
            nc.vector.tensor_scalar(rhs[:, 2:2 + T], thr2[:], mn2[:],
                                    rhs[:, 1:2], Alu.is_gt, Alu.mult)
            # g2[t] = (t < pos) = (thr2 is_lt pos2);  weighted by (1-hn)
            nc.vector.tensor_scalar(rhs[:, 2 + T:2 + 2 * T], thr2[:], pos2,
                                    rhs[:, 0:1], Alu.is_lt, Alu.mult)
            # g3[t] = (min(mn2, t2) < pos2);  weighted by hn
            g3 = smpool.tile([P, T], f32, tag="g3")
            nc.vector.tensor_scalar(g3[:], thr2[:], mn2[:], pos2,
                                    Alu.min, Alu.is_lt)
            nc.vector.tensor_scalar(rhs[:, 2 + 2 * T:2 + 3 * T], g3[:],
                                    rhs[:, 1:2], None, Alu.mult)
            nc.tensor.matmul(acc[:], lhsT[:], rhs[:],
                             start=(g == 0), stop=(g == G - 1))

        # ---- software-pipelined emission (quarter loads, half trees)
        vs = {}
        diffs = {}

        def stage_subtracts(g):
            dfa = dpool.tile([P, HE * D], bf16, tag="diff")
            dfb = dpool.tile([P, HE * D], bf16, tag="diff")
            for q in range(NQ):
                subtract(g, q, vs[(g, q)], dfa if q < 2 else dfb)
            diffs[g] = (dfa, dfb)

        for q in range(NQ):
            vs[(0, q)] = load_q(0, q)
        thr2, ones, big1, mkf_all, neg_all, hn_all = emit_consts()
        if G > 1:
            for q in range(NQ):
                vs[(1, q)] = load_q(1, q)
        for g in range(min(2, G)):
            stage_subtracts(g)
        for g in range(G):
            if g + 2 < G:
                for q in range(NQ):
                    vs[(g + 2, q)] = load_q(g + 2, q)
            dist2 = smpool.tile([P, E], f32, tag="dist2")
            if g == G - 1:
                # last group: quarter trees so the tail after the final DMA
                # byte is as short as possible
                for q in range(NQ):
                    dfh = diffs[g][q // 2]
                    tree(dfh[:, (q % 2) * QE * D:((q % 2) + 1) * QE * D],
                         dist2, q * QE, QE)
            else:
                tree(diffs[g][0][:], dist2, 0, HE)
                tree(diffs[g][1][:], dist2, HE, HE)
            epilogue(g, dist2)
            if g + 2 < G:
                stage_subtracts(g + 2)

        outsb = smpool.tile([2, 2 + 3 * T], f32, tag="outsb")
        nc.vector.tensor_copy(outsb[:], acc[:])
        nc.sync.dma_start(out[:], outsb[:])

    return nc


_CACHE = {}


def _run(in_maps, trace=False):
    from concourse.bass_utils import run_bass_kernel_spmd
    if "nc" not in _CACHE:
        _CACHE["nc"] = build_nc()
    return run_bass_kernel_spmd(_CACHE["nc"], in_maps,
                                core_ids=list(range(NCORES)), trace=trace)


def shard_inputs(predicted, entity_vectors, entity_mask):
    in_maps = []
    for c in range(NCORES):
        s = slice(c * BL, (c + 1) * BL)
        in_maps.append({
            "predicted": np.ascontiguousarray(predicted[s], dtype=np.float32),
            "entity_vectors": np.ascontiguousarray(entity_vectors[s],
                                                   dtype=np.float32),
            "entity_mask": np.ascontiguousarray(entity_mask[s], dtype=np.int32),
        })
    return in_maps


def unshard(results):
    total = np.zeros((2, NC_OUT), dtype=np.float64)
    for r in results:
        total += r["out"].astype(np.float64)
    counts = np.rint(total[:, 0:2]).astype(np.int32)
    incorrect = np.zeros((T, 2, 2), dtype=np.int32)
    incorrect[:, 0, 1] = np.rint(total[0, 2:2 + T]).astype(np.int32)
    incorrect[:, 1, 0] = np.rint(total[1, 2 + T:2 + 2 * T]).astype(np.int32)
    incorrect[:, 1, 1] = np.rint(total[1, 2 + 2 * T:2 + 3 * T]).astype(np.int32)
    return counts, incorrect


def kernel(predicted, entity_vectors, entity_mask):
    res = _run(shard_inputs(predicted, entity_vectors, entity_mask))
    return unshard(res.results)


# revision 41
# speedup vs baseline: 1.1908x; 1.1030x over previous
"""Bass TRN2 kernel for nn_ACCLoss (histogram_binning), 8-core data parallel.

Each of the 8 NeuronCores processes B/8 = 512 samples:
  - streams entity_vectors [512, 64, 256] f32 from HBM as bf16 (cast in DMA),
  - computes squared distances ||v[b,e,:] - predicted[b,:]||^2 via a
    broadcast-subtract (VectorE) + in-place square (ScalarE) + tree-add
    reduction (VectorE),
  - derives per-sample (is_known, has_neg) categories and per-threshold
    incorrect flags from entity_mask (all comparisons done in squared-distance
    space; thresholds squared),
  - reduces the per-sample 0/1 payload columns across the 128-sample
    partition axis with a ones-vector matmul accumulated in PSUM.
Host side sums the 8 per-core count vectors (the "all-reduce" of the
sharding hint — 19 scalars per core — done in numpy at unshard time).
"""

import numpy as np

import concourse.bass as bass
import concourse.mybir as mybir
import concourse.tile as tile
from concourse.vector_clock import ScopedClock

B, E, D = 4096, 64, 256
NCORES = 8
BL = B // NCORES          # samples per core
P = 128                   # samples per group (partition dim)
G = BL // P               # groups per core
THR2 = (0.25, 1.0, 2.25, 4.0, 9.0)   # squared thresholds [0.5,1,1.5,2,3]^2
T = len(THR2)
NC_OUT = 2 + 3 * T        # rhs cols: [1-hn, hn, hn*g1, (1-hn)*g2, hn*g3]
BIGF = float(2 ** 20)

f32 = mybir.dt.float32
bf16 = mybir.dt.bfloat16
i32 = mybir.dt.int32
Alu = mybir.AluOpType
Act = mybir.ActivationFunctionType
AX = mybir.AxisListType


_MAXW = 1  # max sync-waits this walrus build accepts per instruction


class _SplitDrainTC(tile.TileContext):
    """This container's walrus build rejects instructions carrying more
    than one sync-wait ("Too many sync wait commands"); split extra waits
    onto same-engine NoOp carriers emitted just before the instruction."""

    def _commit_and_lower(self, inst, original_block, old_bb_map, bb_to_exit_bb):
        si = getattr(inst, "sync_info", None)
        eng = getattr(inst, "engine", None)
        if si is not None and eng is not None and len(si.on_wait) > _MAXW:
            ow = list(si.on_wait)
            head, tail = ow[:-_MAXW], ow[-_MAXW:]
            while head:
                chunk, head = head[:_MAXW], head[_MAXW:]
                nop = mybir.InstNoOp(
                    name=self.nc.get_next_instruction_name(),
                    sync_info=mybir.SyncInfo(on_wait=chunk, on_update=[]),
                    bass_nofuse=True,
                    engine=eng,
                )
                self._commit_instruction(nop)
            inst.sync_info = mybir.SyncInfo(on_wait=tail, on_update=si.on_update)
        return super()._commit_and_lower(inst, original_block, old_bb_map,
                                         bb_to_exit_bb)

    def _drain_and_barrier(self, tick_clock, wait_clock):
        drain_inst = self.nc.sync.drain()
        wait_clock.add_sem_waits(
            drain_inst.ins, ScopedClock({None: tick_clock.global_clock})
        )
        si = drain_inst.ins.sync_info
        ow = list(si.on_wait) if si is not None else []
        if len(ow) > 1:
            drain_inst.ins.sync_info = mybir.SyncInfo(
                on_wait=ow[:1], on_update=si.on_update
            )
            for w in ow[1:]:
                d = self.nc.sync.drain()
                d.ins.sync_info = mybir.SyncInfo(on_wait=[w], on_update=[])
        self.nc.all_engine_barrier()
        assert self.sems is not None
        popped = self.nc._tile_sem_poison_stack.pop()
        assert popped is self._sem_poison
        self.nc.clear_and_free_semaphores(list(self.sems.allocated().values()))
        self.nc.all_engine_barrier()


def build_nc():
    nc = bass.Bass("TRN2", target_bir_lowering=False, debug=False,
                   num_devices=NCORES)
    pred = nc.dram_tensor("predicted", [BL, D], f32, kind="ExternalInput")
    ev = nc.dram_tensor("entity_vectors", [BL, E, D], f32, kind="ExternalInput")
    em = nc.dram_tensor("entity_mask", [BL, E, 2], i32, kind="ExternalInput")
    out = nc.dram_tensor("out", [2, NC_OUT], f32, kind="ExternalOutput")

    with _SplitDrainTC(nc) as tc, \
         tc.tile_pool(name="v", bufs=8) as vpool, \
         tc.tile_pool(name="diff", bufs=4) as dpool, \
         tc.tile_pool(name="scr", bufs=2) as spool, \
         tc.tile_pool(name="small", bufs=2) as smpool, \
         tc.tile_pool(name="const", bufs=1) as cpool, \
         tc.tile_pool(name="psum", bufs=1, space="PSUM") as ppool:

        acc = ppool.tile([2, NC_OUT], f32, tag="acc")

        # predicted for all groups: one contiguous cast-DMA, queued first
        p_all = cpool.tile([P, G * D], bf16, tag="p_all")
        nc.gpsimd.dma_start(p_all[:].rearrange("p (g d) -> p g d", g=G),
                            pred[:].rearrange("(g p) d -> p g d", g=G))

        NQ = 4           # work items per group
        QE = E // NQ     # entities per work item
        HE = E // 2

        def emit_consts():
            # emitted AFTER the first v DMAs so the Q7 pushes those
            # descriptors before it runs iota; memsets go on DVE
            iota_i = cpool.tile([P, G * E], i32, tag="iota_i")
            nc.gpsimd.iota(iota_i[:].rearrange("p (g e) -> p g e", g=G),
                           [[0, G], [1, E]], channel_multiplier=0)
            iota_f = cpool.tile([P, G * E], f32, tag="iota_f")
            nc.vector.tensor_copy(iota_f[:], iota_i[:])
            iota_mb = cpool.tile([P, G * E], f32, tag="iota_mb")  # iota - BIG
            nc.vector.tensor_scalar(iota_mb[:], iota_f[:], -BIGF, None, Alu.add)
            thr2 = cpool.tile([P, T], f32, tag="thr2")
            for i, t2 in enumerate(THR2):
                nc.vector.memset(thr2[:, i:i + 1], t2)
            ones = cpool.tile([P, 1], f32, tag="ones")
            nc.vector.memset(ones[:], 1.0)
            big1 = cpool.tile([P, 1], f32, tag="big1")
            nc.vector.memset(big1[:], BIGF)

            # mask for all groups: one contiguous DMA on the HWDGE path
            m_all = cpool.tile([P, G * E * 2], i32, tag="m_all")
            nc.sync.dma_start(
                m_all[:].rearrange("p (g e c) -> p g e c", g=G, c=2),
                em[:].rearrange("(g p) e c -> p g e c", g=G))
            m4 = m_all[:].rearrange("p (g e c) -> p g e c", g=G, c=2)

            # batched mask path for ALL groups (independent of distance data)
            mkf_all = cpool.tile([P, G * E], f32, tag="mkf_all")
            nc.vector.tensor_copy(mkf_all[:].rearrange("p (g e) -> p g e", g=G),
                                  m4[:, :, :, 0])
            sel_all = cpool.tile([P, G * E], f32, tag="sel_all")
            nc.vector.tensor_tensor(sel_all[:], mkf_all[:], iota_mb[:], Alu.mult)
            rm_all = cpool.tile([P, G], f32, tag="rm_all")
            nc.vector.tensor_reduce(
                rm_all[:], sel_all[:].rearrange("p (g e) -> p g e", g=G),
                axis=AX.X, op=Alu.min)
            gt_all = cpool.tile([P, G * E], f32, tag="gt_all")
            nc.vector.tensor_tensor(
                gt_all[:].rearrange("p (g e) -> p g e", g=G),
                iota_mb[:].rearrange("p (g e) -> p g e", g=G),
                rm_all[:][:, :, None].broadcast_to([P, G, E]), Alu.is_gt)
            neg_all = cpool.tile([P, G * E], f32, tag="neg_all")
            nc.vector.tensor_tensor(neg_all[:], gt_all[:], mkf_all[:], Alu.mult)
            hn_all = cpool.tile([P, G], f32, tag="hn_all")
            nc.vector.tensor_reduce(
                hn_all[:], neg_all[:].rearrange("p (g e) -> p g e", g=G),
                axis=AX.X, op=Alu.max)
            return thr2, ones, big1, mkf_all, neg_all, hn_all

        def load_q(g, q):
            r0 = g * P
            v = vpool.tile([P, QE * D], bf16, tag="v")
            src = ev[r0:r0 + P, q * QE:(q + 1) * QE, :].rearrange(
                "p e d -> p (e d)")
            nc.gpsimd.dma_start(v[:], src)
            return v

        def subtract(g, q, v, df):
            # quarter subtract into its slice of a half-group diff tile,
            # then in-place square of that slice on ScalarE
            sl = df[:, (q % 2) * QE * D:((q % 2) + 1) * QE * D]
            v3 = v[:].rearrange("p (e d) -> p e d", e=QE)
            d3 = sl.rearrange("p (e d) -> p e d", e=QE)
            p3 = p_all[:, g * D:(g + 1) * D][:, None, :].broadcast_to([P, QE, D])
            nc.vector.tensor_tensor(d3, v3, p3, Alu.subtract)
            nc.scalar.activation(sl, sl, Act.Square)

        def tree(df_ap, dist2, e0, ne):
            # reduce [P, ne, 256] -> dist2[:, e0:e0+ne]
            scrA = spool.tile([P, HE * (D // 2)], bf16, tag="scrA")
            scrB = spool.tile([P, HE * (D // 4)], bf16, tag="scrB")

            def lvl(src_ap, dst_ap, w):
                s3 = src_ap.rearrange("p (e w) -> p e w", e=ne)
                half = w // 2
                nc.vector.tensor_tensor(
                    dst_ap.rearrange("p (e h) -> p e h", e=ne),
                    s3[:, :, 0:half], s3[:, :, half:w], Alu.add)

            cur, w = df_ap, D
            use_a = True
            while w > 2:
                dst_t = scrA if use_a else scrB
                dst = dst_t[:, 0:ne * (w // 2)]
                lvl(cur, dst, w)
                cur, w, use_a = dst, w // 2, not use_a
            lvl(cur, dist2[:, e0:e0 + ne], 2)

        def epilogue(g, dist2):
            neg = neg_all[:, g * E:(g + 1) * E]
            hn = hn_all[:, g:g + 1]
            # min over negatives of dist2:  max(neg*BIG - dist2) = BIG - mn2
            sel2 = smpool.tile([P, E], f32, tag="sel2")
            nc.vector.scalar_tensor_tensor(sel2[:], neg, BIGF, dist2[:],
                                           Alu.mult, Alu.subtract)
            mx = smpool.tile([P, 1], f32, tag="mx")
            nc.vector.reduce_max(mx[:], sel2[:], axis=AX.X)
            mn2 = smpool.tile([P, 1], f32, tag="mn2")
            nc.scalar.activation(mn2[:], mx[:], Act.Identity, bias=big1[:], scale=-1.0)

            k = mkf_all[:, g * E:g * E + 1]
            pos2 = dist2[:, 0:1]
            # lhsT = [1-k, k]; rhs = [1-hn, hn, hn*g1, (1-hn)*g2, hn*g3]
            # PE outer product then gives every count/incorrect cell.
            lhsT = smpool.tile([P, 2], f32, tag="lhsT")
            nc.scalar.activation(lhsT[:, 0:1], k, Act.Identity, bias=ones[:],
                                 scale=-1.0)
            nc.scalar.copy(lhsT[:, 1:2], k)
            rhs = smpool.tile([P, 2 + 3 * T], f32, tag="rhs")
            nc.scalar.activation(rhs[:, 0:1], hn, Act.Identity, bias=ones[:],
                                 scale=-1.0)
            nc.scalar.copy(rhs[:, 1:2], hn)
            # g1[t] = (mn < t) = (thr2 is_gt mn2);  weighted by hn
            nc.vector.tensor_scalar(rhs[:, 2:2 + T], thr2[:], mn2[:],
                                    rhs[:, 1:2], Alu.is_gt, Alu.mult)
            # g2[t] = (t < pos) = (thr2 is_lt pos2);  weighted by (1-hn)
            nc.vector.tensor_scalar(rhs[:, 2 + T:2 + 2 * T], thr2[:], pos2,
                                    rhs[:, 0:1], Alu.is_lt, Alu.mult)
            # g3[t] = (min(mn2, t2) < pos2);  weighted by hn
            g3 = smpool.tile([P, T], f32, tag="g3")
            nc.vector.tensor_scalar(g3[:], thr2[:], mn2[:], pos2,
                                    Alu.min, Alu.is_lt)
            nc.vector.tensor_scalar(rhs[:, 2 + 2 * T:2 + 3 * T], g3[:],
                                    rhs[:, 1:2], None, Alu.mult)
            nc.tensor.matmul(acc[:], lhsT[:], rhs[:],
                             start=(g == 0), stop=(g == G - 1))

        # ---- software-pipelined emission (quarter loads, half trees)
        vs = {}
        diffs = {}

        def stage_subtracts(g):
            dfa = dpool.tile([P, HE * D], bf16, tag="diff")
            dfb = dpool.tile([P, HE * D], bf16, tag="diff")
            for q in range(NQ):
                subtract(g, q, vs[(g, q)], dfa if q < 2 else dfb)
            diffs[g] = (dfa, dfb)

        for q in range(NQ):
            vs[(0, q)] = load_q(0, q)
        thr2, ones, big1, mkf_all, neg_all, hn_all = emit_consts()
        if G > 1:
            for q in range(NQ):
                vs[(1, q)] = load_q(1, q)
        stage_subtracts(0)
        for g in range(G):
            if g + 2 < G:
                for q in range(NQ):
                    vs[(g + 2, q)] = load_q(g + 2, q)
            dist2 = smpool.tile([P, E], f32, tag="dist2")
            if g == G - 1 and G > 1:
                # last group: interleave its subtracts with quarter trees so
                # the tail after the final DMA byte is as short as possible
                dfa = dpool.tile([P, HE * D], bf16, tag="diff")
                dfb = dpool.tile([P, HE * D], bf16, tag="diff")
                subtract(g, 0, vs[(g, 0)], dfa)
                subtract(g, 1, vs[(g, 1)], dfa)
                tree(dfa[:, 0:QE * D], dist2, 0, QE)
                subtract(g, 2, vs[(g, 2)], dfb)
                tree(dfa[:, QE * D:], dist2, QE, QE)
                subtract(g, 3, vs[(g, 3)], dfb)
                tree(dfb[:, 0:QE * D], dist2, 2 * QE, QE)
                tree(dfb[:, QE * D:], dist2, 3 * QE, QE)
            else:
                tree(diffs[g][0][:], dist2, 0, HE)
                tree(diffs[g][1][:], dist2, HE, HE)
            epilogue(g, dist2)
            if g + 1 < G and g + 1 != G - 1:
                stage_subtracts(g + 1)

        outsb = smpool.tile([2, 2 + 3 * T], f32, tag="outsb")
        nc.vector.tensor_copy(outsb[:], acc[:])
        nc.sync.dma_start(out[:], outsb[:])

    return nc


_CACHE = {}


def _run(in_maps, trace=False):
    from concourse.bass_utils import run_bass_kernel_spmd
    if "nc" not in _CACHE:
        _CACHE["nc"] = build_nc()
    return run_bass_kernel_spmd(_CACHE["nc"], in_maps,
                                core_ids=list(range(NCORES)), trace=trace)


def shard_inputs(predicted, entity_vectors, entity_mask):
    in_maps = []
    for c in range(NCORES):
        s = slice(c * BL, (c + 1) * BL)
        in_maps.append({
            "predicted": np.ascontiguousarray(predicted[s], dtype=np.float32),
            "entity_vectors": np.ascontiguousarray(entity_vectors[s],
                                                   dtype=np.float32),
            "entity_mask": np.ascontiguousarray(entity_mask[s], dtype=np.int32),
        })
    return in_maps


def unshard(results):
    total = np.zeros((2, NC_OUT), dtype=np.float64)
    for r in results:
        total += r["out"].astype(np.float64)
    counts = np.rint(total[:, 0:2]).astype(np.int32)
    incorrect = np.zeros((T, 2, 2), dtype=np.int32)
    incorrect[:, 0, 1] = np.rint(total[0, 2:2 + T]).astype(np.int32)
    incorrect[:, 1, 0] = np.rint(total[1, 2 + T:2 + 2 * T]).astype(np.int32)
    incorrect[:, 1, 1] = np.rint(total[1, 2 + 2 * T:2 + 3 * T]).astype(np.int32)
    return counts, incorrect


def kernel(predicted, entity_vectors, entity_mask):
    res = _run(shard_inputs(predicted, entity_vectors, entity_mask))
    return unshard(res.results)
